# revision 1
# baseline (speedup 1.0000x reference)
"""Trainium2 Bass kernel for ChunkedTropicalAttention.

Shards the fused (batch*head) axis over 8 NeuronCores: core i handles batch
i//4 and heads (2*(i%4), 2*(i%4)+1).  Each core computes t=log1p(relu(x)),
tropical (max-plus) q/k/v projections, the chunked tropical attention, expm1,
and a partial out-projection against its 128-column slice of W_out.  The host
sums the four partials per batch (replicated-W_out head parallelism).

Hot-path dtype is fp16 (DVE 2x mode for the scalar-tensor-tensor max/min
accumulations); accumulation of the final projection is fp32 on the PE.
"""

import os
import sys

sys.path.insert(0, "/opt/trn_rl_repo")

import numpy as np

B, S, DM, NH, DK, CH = 2, 512, 512, 8, 64, 128
NCH = S // CH  # 4 query chunks
HPC = 2        # heads per core
NCORES = 8

_prog = None


def _build_program():
    import concourse.bacc as bacc
    import concourse.mybir as mybir
    from concourse.tile import TileContext

    F32 = mybir.dt.float32
    F16 = mybir.dt.float16
    AF = mybir.ActivationFunctionType
    OP = mybir.AluOpType

    nc = bacc.Bacc("TRN2", target_bir_lowering=False, debug=False,
                   num_devices=NCORES)

    xh = nc.dram_tensor("xh", [S, HPC * DK], F32, kind="ExternalInput")
    wcat = nc.dram_tensor("wcat", [1, DK * 3 * DK], F32, kind="ExternalInput")
    wo = nc.dram_tensor("wo", [HPC * DK, DM], F32, kind="ExternalInput")
    outp = nc.dram_tensor("outp", [S, DM], F32, kind="ExternalOutput")

    NW = DK * 3 * DK  # 12288

    with TileContext(nc) as tc:
        with (
            tc.tile_pool(name="const", bufs=1) as cpool,
            tc.tile_pool(name="tt", bufs=4) as tpool,
            tc.tile_pool(name="acc", bufs=8) as apool,
            tc.tile_pool(name="qf", bufs=8) as qpool,
            tc.tile_pool(name="kvt", bufs=2) as kvtpool,
            tc.tile_pool(name="flat", bufs=2) as fpool,
            tc.tile_pool(name="abA", bufs=2) as aapool,
            tc.tile_pool(name="abB", bufs=2) as bbpool,
            tc.tile_pool(name="sc", bufs=8) as scpool,
            tc.tile_pool(name="scr", bufs=2) as scrpool,
            tc.tile_pool(name="ctx", bufs=4) as ctxpool,
            tc.tile_pool(name="proj", bufs=2) as projpool,
            tc.tile_pool(name="ps", bufs=3, space="PSUM") as pspool,
            tc.tile_pool(name="pso", bufs=2, space="PSUM") as psopool,
        ):
            ones = cpool.tile([1, 128], F16, tag="ones")
            nc.vector.memset(ones[:], 1.0)
            wo_sb = cpool.tile([HPC * DK, DM], F32, tag="wo")
            nc.sync.dma_start(wo_sb[:], wo[:])

            # t = log1p(relu(x)) as 4 fp32 s-tiles [128, 128]
            t_tiles = []
            for st in range(NCH):
                xt_ = tpool.tile([CH, HPC * DK], F32, tag="t")
                nc.sync.dma_start(xt_[:], xh[st * CH:(st + 1) * CH, :])
                nc.vector.tensor_scalar(xt_[:], xt_[:], 0.0, None, OP.max)
                nc.scalar.activation(xt_[:], xt_[:], AF.Ln, bias=1.0, scale=1.0)
                t_tiles.append(xt_)

            # Wb: wcat broadcast across partitions, fp16 [128, 12288]
            qfs = {}
            kvts = {}
            with tc.tile_pool(name="wb", bufs=1) as wbpool:
                wb = wbpool.tile([128, NW], F16, tag="Wb")
                for wch in range(3):
                    wflat = fpool.tile([1, 8 * S], F16, tag="flat")
                    nc.gpsimd.dma_start(
                        wflat[:], wcat[:, wch * 4096:(wch + 1) * 4096])
                    for j in range(8):
                        ps = pspool.tile([128, 512], F32, tag="ps")
                        nc.tensor.matmul(ps[:], ones[:],
                                         wflat[:, j * 512:(j + 1) * 512])
                        nc.scalar.copy(
                            wb[:, wch * 4096 + j * 512: wch * 4096 + (j + 1) * 512],
                            ps[:])

                # tropical linears:
                # acc[h,st][c, w*64+o] = max_i(W_w[o,i] + t[c, h*64+i])
                for h in range(HPC):
                    for st in range(NCH):
                        acc = apool.tile([CH, 3 * DK], F16, tag="acc")
                        for i in range(DK):
                            wbi = wb[:, i * 192:(i + 1) * 192]
                            tcol = t_tiles[st][:, h * DK + i: h * DK + i + 1]
                            if i == 0:
                                nc.vector.tensor_scalar(acc[:], wbi, tcol, None,
                                                        OP.add)
                            else:
                                nc.vector.scalar_tensor_tensor(
                                    acc[:], wbi, tcol, acc[:], OP.add, OP.max)
                        qf = qpool.tile([CH, DK], F32, tag="qf")
                        nc.scalar.copy(qf[:], acc[:, 0:DK])
                        qfs[h, st] = qf
                        if st == 0:
                            kvt_h = kvtpool.tile([128, 512], F16, tag="kvt")
                            kvts[h] = kvt_h
                        nc.sync.dma_start(
                            kvts[h][:, st * CH:(st + 1) * CH],
                            acc[:, DK:3 * DK], transpose=True)

            def build_bcast(h, row0):
                """Broadcast rows [row0, row0+64) of the kvT tile (kT or vT)
                across all 128 partitions -> [128, 64*S] fp16."""
                big = bigpool.tile([128, DK * S], F16, tag="big")
                for j in range(8):
                    flat = fpool.tile([1, 8 * S], F16, tag="flat")
                    nc.sync.dma_start(
                        flat[:], kvts[h][row0 + 8 * j: row0 + 8 * j + 8, :])
                    for half in range(4):
                        d = 8 * j + 2 * half
                        ps = pspool.tile([128, 2 * S], F32, tag="ps")
                        nc.tensor.matmul(ps[:, 0:S], ones[:],
                                         flat[:, 2 * half * S:(2 * half + 1) * S])
                        nc.tensor.matmul(ps[:, S:2 * S], ones[:],
                                         flat[:, (2 * half + 1) * S:(2 * half + 2) * S])
                        nc.scalar.copy(big[:, d * S:(d + 2) * S], ps[:])
                return big

            ctxpairs = []
            for _ch in range(NCH):
                ctxp = ctxpool.tile([CH, HPC * DK], F16, tag="ctxp")
                ctxpairs.append(ctxp)
            scores_tiles = {}
            _bigcm = tc.tile_pool(name="big", bufs=2)
            bigpool = _bigcm.__enter__()
            for h in range(HPC):
                kb = build_bcast(h, 0)      # kT broadcast
                # stage 1: A = max_d(k-q), Bt = min_d(k-q); scores = Bt - A
                for ch in range(NCH):
                    A = aapool.tile([CH, S], F16, tag="A")
                    Bt = bbpool.tile([CH, S], F16, tag="B")
                    qf = qfs[h, ch]
                    nc.vector.tensor_scalar(A[:], kb[:, 0:S], qf[:, 0:1], None,
                                            OP.subtract)
                    nc.vector.tensor_scalar(Bt[:], kb[:, 0:S], qf[:, 0:1], None,
                                            OP.subtract)
                    for d in range(1, DK):
                        kbd = kb[:, d * S:(d + 1) * S]
                        qcol = qf[:, d:d + 1]
                        nc.vector.scalar_tensor_tensor(
                            A[:], kbd, qcol, A[:], OP.subtract, OP.max)
                        nc.vector.scalar_tensor_tensor(
                            Bt[:], kbd, qcol, Bt[:], OP.subtract, OP.min)
                    sc = scpool.tile([CH, S], F16, tag="sc")
                    nc.vector.tensor_tensor(sc[:], Bt[:], A[:], OP.subtract)
                    scores_tiles[h, ch] = sc

                vb = build_bcast(h, DK)     # vT broadcast
                # stage 2: ctx[c, e] = max_s(scores[c,s] + v[s,e])
                # (tensor_tensor_reduce crashes TRN2 here; use TT add +
                #  tensor_reduce max instead)
                for ch in range(NCH):
                    sc = scores_tiles[h, ch]
                    for e in range(DK):
                        scr = scrpool.tile([CH, S], F16, tag="scr")
                        nc.vector.tensor_tensor(
                            scr[:], sc[:], vb[:, e * S:(e + 1) * S], OP.add)
                        nc.vector.tensor_reduce(
                            ctxpairs[ch][:, h * DK + e: h * DK + e + 1],
                            scr[:], axis=mybir.AxisListType.X, op=OP.max)

            _bigcm.__exit__(None, None, None)
            # projection: outp[ch] = (exp(ctx)-1).T-matmul with wo
            for ch in range(NCH):
                eT = projpool.tile([128, 128], F16, tag="eT")
                nc.sync.dma_start(eT[:], ctxpairs[ch][:], transpose=True)
                ex = projpool.tile([128, 128], F32, tag="ex")
                nc.scalar.activation(ex[:], eT[:], AF.Exp)
                nc.vector.tensor_scalar(ex[:], ex[:], -1.0, None, OP.add)
                pso = psopool.tile([128, DM], F32, tag="pso")
                nc.tensor.matmul(pso[:], ex[:], wo_sb[:])
                osb = projpool.tile([128, DM], F32, tag="osb")
                nc.scalar.copy(osb[:], pso[:])
                nc.sync.dma_start(outp[ch * CH:(ch + 1) * CH, :], osb[:])

    nc.compile()
    return nc


def _core_inputs(x, Wq, Wk, Wv, W_out, core):
    b, hp = divmod(core, 4)
    h0 = 2 * hp
    sl = slice(DK * h0, DK * h0 + HPC * DK)
    xh = np.ascontiguousarray(x[b, :, sl], dtype=np.float32)
    wcat = np.ascontiguousarray(
        np.concatenate([Wq.T, Wk.T, Wv.T], axis=1), dtype=np.float32
    ).reshape(1, -1)
    wo = np.ascontiguousarray(W_out[:, sl].T, dtype=np.float32)
    return {"xh": xh, "wcat": wcat, "wo": wo}


_runner = None


def _make_runner(nc):
    """Build the shard_map-jitted executable ONCE (mirrors the multi-core
    path of bass2jax.run_bass_via_pjrt) so repeat calls skip re-tracing."""
    import jax
    import numpy as _np
    from concourse import bass2jax, mybir
    from concourse.bass2jax import (
        Mesh, PartitionSpec, _bass_exec_p, install_neuronx_cc_hook,
        partition_id_tensor, shard_map,
    )

    install_neuronx_cc_hook()
    partition_name = (nc.partition_id_tensor.name
                      if nc.partition_id_tensor else None)
    in_names, out_names, out_avals, zero_outs = [], [], [], []
    for alloc in nc.m.functions[0].allocations:
        if not isinstance(alloc, mybir.MemoryLocationSet):
            continue
        name = alloc.memorylocations[0].name
        if alloc.kind == "ExternalInput":
            if name != partition_name:
                in_names.append(name)
        elif alloc.kind == "ExternalOutput":
            shape = tuple(alloc.tensor_shape)
            dtype = mybir.dt.np(alloc.dtype)
            out_avals.append(jax.core.ShapedArray(shape, dtype))
            out_names.append(name)
            zero_outs.append(_np.zeros(shape, dtype))
    n_params = len(in_names)
    n_outs = len(out_avals)
    all_names = list(in_names) + list(out_names)
    if partition_name is not None:
        all_names.append(partition_name)

    def _body(*args):
        operands = list(args)
        if partition_name is not None:
            operands.append(partition_id_tensor())
        return tuple(_bass_exec_p.bind(
            *operands, out_avals=tuple(out_avals), in_names=tuple(all_names),
            out_names=tuple(out_names), lowering_input_output_aliases=(),
            sim_require_finite=True, sim_require_nnan=True, nc=nc))

    devices = jax.devices()[:NCORES]
    mesh = Mesh(_np.asarray(devices), ("core",))
    in_specs = (PartitionSpec("core"),) * (n_params + n_outs)
    out_specs = (PartitionSpec("core"),) * n_outs
    donate = tuple(range(n_params, n_params + n_outs))
    sharded = jax.jit(
        shard_map(_body, mesh=mesh, in_specs=in_specs, out_specs=out_specs,
                  check_rep=False),
        donate_argnums=donate, keep_unused=True)


    def run(in_maps, fn=None):
        per_core = [[_np.asarray(m[nm]) for nm in in_names] for m in in_maps]
        concat_in = [
            _np.concatenate([per_core[c][i] for c in range(NCORES)], axis=0)
            for i in range(n_params)]
        concat_zeros = [
            _np.zeros((NCORES * z.shape[0], *z.shape[1:]), z.dtype)
            for z in zero_outs]
        out_arrs = (fn or sharded)(*concat_in, *concat_zeros)
        return [
            {nm: _np.asarray(out_arrs[i]).reshape(NCORES, *out_avals[i].shape)[c]
             for i, nm in enumerate(out_names)}
            for c in range(NCORES)]

    return run


def kernel(x, Wq, Wk, Wv, W_out):
    global _prog
    x = np.asarray(x, dtype=np.float32)
    Wq = np.asarray(Wq, dtype=np.float32)
    Wk = np.asarray(Wk, dtype=np.float32)
    Wv = np.asarray(Wv, dtype=np.float32)
    W_out = np.asarray(W_out, dtype=np.float32)

    global _runner
    if _prog is None:
        _prog = _build_program()
    if _runner is None:
        _runner = _make_runner(_prog)

    in_maps = [_core_inputs(x, Wq, Wk, Wv, W_out, c) for c in range(NCORES)]
    results = _runner(in_maps)
    kernel._last = None

    out = np.zeros((B, S, DM), dtype=np.float32)
    for c in range(NCORES):
        out[c // 4] += results[c]["outp"]
    return out


def time_device(x, Wq, Wk, Wv, W_out, n=5):
    """Differential device-time estimate: min over n of t(2 chained execs)
    minus min over n of t(1 exec)."""
    import time as _t
    global _prog, _runner
    if _prog is None:
        _prog = _build_program()
    if _runner is None:
        _runner = _make_runner(_prog)
    in_maps = [_core_inputs(np.asarray(x, np.float32), np.asarray(Wq, np.float32),
                            np.asarray(Wk, np.float32), np.asarray(Wv, np.float32),
                            np.asarray(W_out, np.float32), c)
               for c in range(NCORES)]
    _runner(in_maps)  # warm
    t1 = []
    for _ in range(n):
        t0 = _t.perf_counter()
        _runner(in_maps)
        t1.append(_t.perf_counter() - t0)
    return min(t1) * 1e9, min(t1) * 1e9



# revision 2
# speedup vs baseline: 3.0901x; 3.0901x over previous
"""Trainium2 Bass kernel for ChunkedTropicalAttention.

Shards the fused (batch*head) axis over 8 NeuronCores: core i handles batch
i//4 and heads (2*(i%4), 2*(i%4)+1).  Each core computes t=log1p(relu(x)),
tropical (max-plus) q/k/v projections, the chunked tropical attention, expm1,
and a partial out-projection against its 128-row slice of W_out^T.  The four
partials per batch are summed ON DEVICE with a fp16 ReduceScatter over the
core groups {0..3} / {4..7}; each core then returns only its 128-row slice of
the final output.

The wall-clock of a full call is dominated by the axon tunnel (~70ms RTT,
~100MB/s), so all kernel I/O is fp16 (2.2MB up, 1MB down) and the XLA output
placeholder buffers are persistent device arrays (no per-call upload, no
donation) instead of host-built zeros.
"""

import sys

sys.path.insert(0, "/opt/trn_rl_repo")

import numpy as np

B, S, DM, NH, DK, CH = 2, 512, 512, 8, 64, 128
NCH = S // CH  # 4 query chunks
HPC = 2        # heads per core
NCORES = 8

_prog = None


def _build_program():
    import concourse.bacc as bacc
    import concourse.mybir as mybir
    from concourse.tile import TileContext

    F32 = mybir.dt.float32
    F16 = mybir.dt.float16
    AF = mybir.ActivationFunctionType
    OP = mybir.AluOpType

    nc = bacc.Bacc("TRN2", target_bir_lowering=False, debug=False,
                   num_devices=NCORES)

    xh = nc.dram_tensor("xh", [S, HPC * DK], F16, kind="ExternalInput")
    wcat = nc.dram_tensor("wcat", [1, DK * 3 * DK], F16, kind="ExternalInput")
    wo = nc.dram_tensor("wo", [HPC * DK, DM], F16, kind="ExternalInput")
    outp = nc.dram_tensor("outp", [CH, DM], F16, kind="ExternalOutput")

    NW = DK * 3 * DK  # 12288

    with TileContext(nc) as tc:
        with (
            tc.tile_pool(name="const", bufs=1) as cpool,
            tc.tile_pool(name="x16", bufs=2) as xpool,
            tc.tile_pool(name="tt", bufs=4) as tpool,
            tc.tile_pool(name="acc", bufs=8) as apool,
            tc.tile_pool(name="qf", bufs=8) as qpool,
            tc.tile_pool(name="kvt", bufs=2) as kvtpool,
            tc.tile_pool(name="flat", bufs=2) as fpool,
            tc.tile_pool(name="abA", bufs=2) as aapool,
            tc.tile_pool(name="abB", bufs=2) as bbpool,
            tc.tile_pool(name="sc", bufs=8) as scpool,
            tc.tile_pool(name="scr", bufs=2) as scrpool,
            tc.tile_pool(name="ctx", bufs=4) as ctxpool,
            tc.tile_pool(name="proj", bufs=2) as projpool,
            tc.tile_pool(name="ps", bufs=3, space="PSUM") as pspool,
            tc.tile_pool(name="pso", bufs=2, space="PSUM") as psopool,
            tc.tile_pool(name="dramcc", bufs=1, space="DRAM") as dpool,
        ):
            ones = cpool.tile([1, 128], F16, tag="ones")
            nc.vector.memset(ones[:], 1.0)
            wo_sb = cpool.tile([HPC * DK, DM], F16, tag="wo")
            nc.sync.dma_start(wo_sb[:], wo[:])

            # t = log1p(relu(x)) as 4 fp32 s-tiles [128, 128]
            t_tiles = []
            for st in range(NCH):
                x16 = xpool.tile([CH, HPC * DK], F16, tag="x16")
                nc.sync.dma_start(x16[:], xh[st * CH:(st + 1) * CH, :])
                nc.vector.tensor_scalar(x16[:], x16[:], 0.0, None, OP.max)
                t32 = tpool.tile([CH, HPC * DK], F32, tag="t")
                nc.scalar.activation(t32[:], x16[:], AF.Ln, bias=1.0, scale=1.0)
                t_tiles.append(t32)

            # Wb: wcat broadcast across partitions, fp16 [128, 12288]
            qfs = {}
            kvts = {}
            with tc.tile_pool(name="wb", bufs=1) as wbpool:
                wb = wbpool.tile([128, NW], F16, tag="Wb")
                for wch in range(3):
                    wflat = fpool.tile([1, 8 * S], F16, tag="flat")
                    nc.gpsimd.dma_start(
                        wflat[:], wcat[:, wch * 4096:(wch + 1) * 4096])
                    for j in range(8):
                        ps = pspool.tile([128, 512], F32, tag="ps")
                        nc.tensor.matmul(ps[:], ones[:],
                                         wflat[:, j * 512:(j + 1) * 512])
                        nc.scalar.copy(
                            wb[:, wch * 4096 + j * 512: wch * 4096 + (j + 1) * 512],
                            ps[:])

                # tropical linears:
                # acc[h,st][c, w*64+o] = max_i(W_w[o,i] + t[c, h*64+i])
                for h in range(HPC):
                    for st in range(NCH):
                        acc = apool.tile([CH, 3 * DK], F16, tag="acc")
                        for i in range(DK):
                            wbi = wb[:, i * 192:(i + 1) * 192]
                            tcol = t_tiles[st][:, h * DK + i: h * DK + i + 1]
                            if i == 0:
                                nc.vector.tensor_scalar(acc[:], wbi, tcol, None,
                                                        OP.add)
                            else:
                                nc.vector.scalar_tensor_tensor(
                                    acc[:], wbi, tcol, acc[:], OP.add, OP.max)
                        qf = qpool.tile([CH, DK], F32, tag="qf")
                        nc.scalar.copy(qf[:], acc[:, 0:DK])
                        qfs[h, st] = qf
                        if st == 0:
                            kvt_h = kvtpool.tile([128, 512], F16, tag="kvt")
                            kvts[h] = kvt_h
                        nc.sync.dma_start(
                            kvts[h][:, st * CH:(st + 1) * CH],
                            acc[:, DK:3 * DK], transpose=True)

            def build_bcast(h, row0):
                """Broadcast rows [row0, row0+64) of the kvT tile (kT or vT)
                across all 128 partitions -> [128, 64*S] fp16."""
                big = bigpool.tile([128, DK * S], F16, tag="big")
                for j in range(8):
                    flat = fpool.tile([1, 8 * S], F16, tag="flat")
                    nc.sync.dma_start(
                        flat[:], kvts[h][row0 + 8 * j: row0 + 8 * j + 8, :])
                    for half in range(4):
                        d = 8 * j + 2 * half
                        ps = pspool.tile([128, 2 * S], F32, tag="ps")
                        nc.tensor.matmul(ps[:, 0:S], ones[:],
                                         flat[:, 2 * half * S:(2 * half + 1) * S])
                        nc.tensor.matmul(ps[:, S:2 * S], ones[:],
                                         flat[:, (2 * half + 1) * S:(2 * half + 2) * S])
                        nc.scalar.copy(big[:, d * S:(d + 2) * S], ps[:])
                return big

            ctxpairs = []
            for _ch in range(NCH):
                ctxp = ctxpool.tile([CH, HPC * DK], F16, tag="ctxp")
                ctxpairs.append(ctxp)
            scores_tiles = {}
            _bigcm = tc.tile_pool(name="big", bufs=2)
            bigpool = _bigcm.__enter__()
            for h in range(HPC):
                kb = build_bcast(h, 0)      # kT broadcast
                # stage 1: A = max_d(k-q), Bt = min_d(k-q); scores = Bt - A
                for ch in range(NCH):
                    A = aapool.tile([CH, S], F16, tag="A")
                    Bt = bbpool.tile([CH, S], F16, tag="B")
                    qf = qfs[h, ch]
                    nc.vector.tensor_scalar(A[:], kb[:, 0:S], qf[:, 0:1], None,
                                            OP.subtract)
                    nc.vector.tensor_scalar(Bt[:], kb[:, 0:S], qf[:, 0:1], None,
                                            OP.subtract)
                    for d in range(1, DK):
                        kbd = kb[:, d * S:(d + 1) * S]
                        qcol = qf[:, d:d + 1]
                        nc.vector.scalar_tensor_tensor(
                            A[:], kbd, qcol, A[:], OP.subtract, OP.max)
                        nc.vector.scalar_tensor_tensor(
                            Bt[:], kbd, qcol, Bt[:], OP.subtract, OP.min)
                    sc = scpool.tile([CH, S], F16, tag="sc")
                    nc.vector.tensor_tensor(sc[:], Bt[:], A[:], OP.subtract)
                    scores_tiles[h, ch] = sc

                vb = build_bcast(h, DK)     # vT broadcast
                # stage 2: ctx[c, e] = max_s(scores[c,s] + v[s,e])
                # (tensor_tensor_reduce crashes TRN2 here; use TT add +
                #  tensor_reduce max instead)
                for ch in range(NCH):
                    sc = scores_tiles[h, ch]
                    for e in range(DK):
                        scr = scrpool.tile([CH, S], F16, tag="scr")
                        nc.vector.tensor_tensor(
                            scr[:], sc[:], vb[:, e * S:(e + 1) * S], OP.add)
                        nc.vector.tensor_reduce(
                            ctxpairs[ch][:, h * DK + e: h * DK + e + 1],
                            scr[:], axis=mybir.AxisListType.X, op=OP.max)

            _bigcm.__exit__(None, None, None)
            # projection: partial[ch] = (exp(ctx)-1).T-matmul with wo, then
            # fp16 ReduceScatter over the 4-core batch group -> this core's
            # 128-row slice of the final output.
            partial = dpool.tile([S, DM], F16, tag="partial")
            red = dpool.tile([CH, DM], F16, tag="red")
            for ch in range(NCH):
                eT = projpool.tile([128, 128], F16, tag="eT")
                nc.sync.dma_start(eT[:], ctxpairs[ch][:], transpose=True)
                ex = projpool.tile([128, 128], F16, tag="ex")
                nc.scalar.activation(ex[:], eT[:], AF.Exp)
                nc.vector.tensor_scalar(ex[:], ex[:], -1.0, None, OP.add)
                pso = psopool.tile([128, DM], F32, tag="pso")
                nc.tensor.matmul(pso[:], ex[:], wo_sb[:])
                osb = projpool.tile([128, DM], F16, tag="osb")
                nc.scalar.copy(osb[:], pso[:])
                nc.sync.dma_start(partial[ch * CH:(ch + 1) * CH, :], osb[:])
            nc.gpsimd.collective_compute(
                "ReduceScatter",
                mybir.AluOpType.add,
                replica_groups=[[0, 1, 2, 3], [4, 5, 6, 7]],
                ins=[partial.opt()],
                outs=[red.opt()],
            )
            nc.gpsimd.dma_start(outp[:], red[:])

    nc.compile()
    return nc


def _prep_host(x, Wq, Wk, Wv, W_out):
    """fp16 host-side packing shared across cores."""
    x16 = np.asarray(x, np.float32).astype(np.float16)          # (B,S,DM)
    wcat16 = np.ascontiguousarray(
        np.concatenate([np.asarray(Wq).T, np.asarray(Wk).T, np.asarray(Wv).T],
                       axis=1), dtype=np.float16).reshape(1, -1)
    wot16 = np.asarray(W_out, np.float32).T.astype(np.float16)  # (DM, DM)
    return x16, wcat16, wot16


def _core_inputs(x16, wcat16, wot16, core):
    b, hp = divmod(core, 4)
    sl = slice(DK * 2 * hp, DK * 2 * hp + HPC * DK)
    return {
        "xh": np.ascontiguousarray(x16[b, :, sl]),
        "wcat": wcat16,
        "wo": np.ascontiguousarray(wot16[sl, :]),
    }


_runner = None


def _make_runner(nc):
    """Build the shard_map-jitted executable ONCE (mirrors the multi-core
    path of bass2jax.run_bass_via_pjrt) so repeat calls skip re-tracing.

    Output placeholder operands are persistent device-resident arrays and are
    NOT donated: the NEFF writes every element of its outputs, so the
    placeholder content is never read and no per-call host->device upload of
    zero buffers is needed."""
    import jax
    import numpy as _np
    from concourse import mybir
    from concourse.bass2jax import (
        Mesh, PartitionSpec, _bass_exec_p, install_neuronx_cc_hook,
        partition_id_tensor, shard_map,
    )

    install_neuronx_cc_hook()
    partition_name = (nc.partition_id_tensor.name
                      if nc.partition_id_tensor else None)
    in_names, out_names, out_avals, zero_outs = [], [], [], []
    for alloc in nc.m.functions[0].allocations:
        if not isinstance(alloc, mybir.MemoryLocationSet):
            continue
        name = alloc.memorylocations[0].name
        if alloc.kind == "ExternalInput":
            if name != partition_name:
                in_names.append(name)
        elif alloc.kind == "ExternalOutput":
            shape = tuple(alloc.tensor_shape)
            dtype = mybir.dt.np(alloc.dtype)
            out_avals.append(jax.core.ShapedArray(shape, dtype))
            out_names.append(name)
            zero_outs.append(_np.zeros(shape, dtype))
    n_params = len(in_names)
    all_names = list(in_names) + list(out_names)
    if partition_name is not None:
        all_names.append(partition_name)

    def _body(*args):
        operands = list(args)
        if partition_name is not None:
            operands.append(partition_id_tensor())
        return tuple(_bass_exec_p.bind(
            *operands, out_avals=tuple(out_avals), in_names=tuple(all_names),
            out_names=tuple(out_names), lowering_input_output_aliases=(),
            sim_require_finite=True, sim_require_nnan=True, nc=nc))

    devices = jax.devices()[:NCORES]
    mesh = Mesh(_np.asarray(devices), ("core",))
    n_outs = len(out_avals)
    in_specs = (PartitionSpec("core"),) * (n_params + n_outs)
    out_specs = (PartitionSpec("core"),) * n_outs
    sharded = jax.jit(
        shard_map(_body, mesh=mesh, in_specs=in_specs, out_specs=out_specs,
                  check_rep=False),
        keep_unused=True)

    # one-time upload of the (never-read) output placeholder buffers
    out_sharding = jax.sharding.NamedSharding(mesh, PartitionSpec("core"))
    placeholders = [
        jax.device_put(
            _np.zeros((NCORES * z.shape[0], *z.shape[1:]), z.dtype),
            out_sharding)
        for z in zero_outs]
    jax.block_until_ready(placeholders)

    def run(in_maps):
        per_core = [[_np.asarray(m[nm]) for nm in in_names] for m in in_maps]
        concat_in = [
            _np.concatenate([per_core[c][i] for c in range(NCORES)], axis=0)
            for i in range(n_params)]
        out_arrs = sharded(*concat_in, *placeholders)
        return [
            {nm: _np.asarray(out_arrs[i]).reshape(NCORES, *out_avals[i].shape)[c]
             for i, nm in enumerate(out_names)}
            for c in range(NCORES)]

    return run


def kernel(x, Wq, Wk, Wv, W_out):
    global _prog, _runner
    if _prog is None:
        _prog = _build_program()
    if _runner is None:
        _runner = _make_runner(_prog)

    x16, wcat16, wot16 = _prep_host(x, Wq, Wk, Wv, W_out)
    in_maps = [_core_inputs(x16, wcat16, wot16, c) for c in range(NCORES)]
    results = _runner(in_maps)

    out = np.empty((B, S, DM), dtype=np.float32)
    for c in range(NCORES):
        b, hp = divmod(c, 4)
        out[b, hp * CH:(hp + 1) * CH, :] = results[c]["outp"]
    return out


def time_device(x, Wq, Wk, Wv, W_out, n=5):
    """Min wall-clock of a warm full call (prep + upload + exec + fetch)."""
    import time as _t
    global _prog, _runner
    if _prog is None:
        _prog = _build_program()
    if _runner is None:
        _runner = _make_runner(_prog)
    x16, wcat16, wot16 = _prep_host(x, Wq, Wk, Wv, W_out)
    in_maps = [_core_inputs(x16, wcat16, wot16, c) for c in range(NCORES)]
    _runner(in_maps)  # warm
    t1 = []
    for _ in range(n):
        t0 = _t.perf_counter()
        _runner(in_maps)
        t1.append(_t.perf_counter() - t0)
    return min(t1) * 1e9, min(t1) * 1e9


# revision 8
# speedup vs baseline: 3.8923x; 1.2596x over previous
"""Trainium2 Bass kernel for ChunkedTropicalAttention.

Shards the fused (batch*head) axis over 8 NeuronCores: core i handles batch
i//4 and heads (2*(i%4), 2*(i%4)+1).  Each core computes t=log1p(relu(x)),
tropical (max-plus) q/k/v projections, the chunked tropical attention, expm1,
and a partial out-projection against its 128-row slice of W_out^T.  The four
partials per batch are summed ON DEVICE with a fp16 ReduceScatter over the
core groups {0..3} / {4..7}; each core then returns only its 128-row slice of
the final output.

The wall-clock of a full call is dominated by the axon tunnel (~70ms RTT,
~100MB/s), so all kernel I/O is fp16 (2.2MB up, 1MB down) and the XLA output
placeholder buffers are persistent device arrays (no per-call upload, no
donation) instead of host-built zeros.
"""

import sys

sys.path.insert(0, "/opt/trn_rl_repo")

import numpy as np

B, S, DM, NH, DK, CH = 2, 512, 512, 8, 64, 128
NCH = S // CH  # 4 query chunks
HPC = 2        # heads per core
NCORES = 8

_prog = None


def _build_program():
    import concourse.bacc as bacc
    import concourse.mybir as mybir
    from concourse.tile import TileContext

    F32 = mybir.dt.float32
    F16 = mybir.dt.float16
    AF = mybir.ActivationFunctionType
    OP = mybir.AluOpType

    nc = bacc.Bacc("TRN2", target_bir_lowering=False, debug=False,
                   num_devices=NCORES)

    NW = DK * 3 * DK  # 12288

    # Distributed weight upload: each core carries 1/8 of wcat and 64 of the
    # 128 W_out^T rows its batch-group needs; AllGathers reconstruct them.
    xh = nc.dram_tensor("xh", [S, HPC * DK], F16, kind="ExternalInput")
    wcat_in = nc.dram_tensor("wcat_in", [1, NW // NCORES], F16,
                             kind="ExternalInput")
    wo_in = nc.dram_tensor("wo_in", [DK, DM], F16, kind="ExternalInput")
    outp = nc.dram_tensor("outp", [CH, DM], F16, kind="ExternalOutput")

    with TileContext(nc) as tc:
        with (
            tc.tile_pool(name="const", bufs=1) as cpool,
            tc.tile_pool(name="x16", bufs=2) as xpool,
            tc.tile_pool(name="tt", bufs=4) as tpool,
            tc.tile_pool(name="acc", bufs=8) as apool,
            tc.tile_pool(name="qf", bufs=8) as qpool,
            tc.tile_pool(name="kvt", bufs=2) as kvtpool,
            tc.tile_pool(name="flat", bufs=2) as fpool,
            tc.tile_pool(name="abA", bufs=2) as aapool,
            tc.tile_pool(name="abB", bufs=2) as bbpool,
            tc.tile_pool(name="sc", bufs=8) as scpool,
            tc.tile_pool(name="scr", bufs=2) as scrpool,
            tc.tile_pool(name="ctx", bufs=4) as ctxpool,
            tc.tile_pool(name="proj", bufs=2) as projpool,
            tc.tile_pool(name="ps", bufs=3, space="PSUM") as pspool,
            tc.tile_pool(name="pso", bufs=2, space="PSUM") as psopool,
            tc.tile_pool(name="dramcc", bufs=1, space="DRAM") as dpool,
        ):
            # gather the distributed weights first so the collectives overlap
            # nothing (they're first) and everything downstream reads bounces
            wcat_b = dpool.tile([1, NW // NCORES], F16, tag="wcat_b")
            wcat = dpool.tile([1, NW], F16, tag="wcat")
            nc.gpsimd.dma_start(wcat_b[:], wcat_in[:])
            nc.gpsimd.collective_compute(
                "AllGather", mybir.AluOpType.bypass,
                replica_groups=[[0, 1, 2, 3, 4, 5, 6, 7]],
                ins=[wcat_b.opt()], outs=[wcat.opt()])
            wo_b = dpool.tile([DK, DM], F16, tag="wo_b")
            wo_full = dpool.tile([HPC * DK, DM], F16, tag="wo_full")
            nc.gpsimd.dma_start(wo_b[:], wo_in[:])
            nc.gpsimd.collective_compute(
                "AllGather", mybir.AluOpType.bypass,
                replica_groups=[[0, 4], [1, 5], [2, 6], [3, 7]],
                ins=[wo_b.opt()], outs=[wo_full.opt()])

            ones = cpool.tile([1, 128], F16, tag="ones")
            nc.vector.memset(ones[:], 1.0)
            wo_sb = cpool.tile([HPC * DK, DM], F16, tag="wo")
            nc.sync.dma_start(wo_sb[:], wo_full[:])

            # t = log1p(relu(x)) as 4 fp32 s-tiles [128, 128]
            t_tiles = []
            for st in range(NCH):
                x16 = xpool.tile([CH, HPC * DK], F16, tag="x16")
                nc.sync.dma_start(x16[:], xh[st * CH:(st + 1) * CH, :])
                nc.vector.tensor_scalar(x16[:], x16[:], 0.0, None, OP.max)
                t32 = tpool.tile([CH, HPC * DK], F32, tag="t")
                nc.scalar.activation(t32[:], x16[:], AF.Ln, bias=1.0, scale=1.0)
                t_tiles.append(t32)

            # Wb: wcat broadcast across partitions, fp16 [128, 12288]
            qfs = {}
            kvts = {}
            with tc.tile_pool(name="wb", bufs=1) as wbpool:
                wb = wbpool.tile([128, NW], F16, tag="Wb")
                for wch in range(3):
                    wflat = fpool.tile([1, 8 * S], F16, tag="flat")
                    nc.gpsimd.dma_start(
                        wflat[:], wcat[:, wch * 4096:(wch + 1) * 4096])
                    for j in range(8):
                        ps = pspool.tile([128, 512], F32, tag="ps")
                        nc.tensor.matmul(ps[:], ones[:],
                                         wflat[:, j * 512:(j + 1) * 512])
                        nc.scalar.copy(
                            wb[:, wch * 4096 + j * 512: wch * 4096 + (j + 1) * 512],
                            ps[:])

                # tropical linears:
                # acc[h,st][c, w*64+o] = max_i(W_w[o,i] + t[c, h*64+i])
                for h in range(HPC):
                    for st in range(NCH):
                        acc = apool.tile([CH, 3 * DK], F16, tag="acc")
                        for i in range(DK):
                            wbi = wb[:, i * 192:(i + 1) * 192]
                            tcol = t_tiles[st][:, h * DK + i: h * DK + i + 1]
                            if i == 0:
                                nc.vector.tensor_scalar(acc[:], wbi, tcol, None,
                                                        OP.add)
                            else:
                                nc.vector.scalar_tensor_tensor(
                                    acc[:], wbi, tcol, acc[:], OP.add, OP.max)
                        qf = qpool.tile([CH, DK], F32, tag="qf")
                        nc.scalar.copy(qf[:], acc[:, 0:DK])
                        qfs[h, st] = qf
                        if st == 0:
                            kvt_h = kvtpool.tile([128, 512], F16, tag="kvt")
                            kvts[h] = kvt_h
                        nc.sync.dma_start(
                            kvts[h][:, st * CH:(st + 1) * CH],
                            acc[:, DK:3 * DK], transpose=True)

            def build_bcast(h, row0):
                """Broadcast rows [row0, row0+64) of the kvT tile (kT or vT)
                across all 128 partitions -> [128, 64*S] fp16."""
                big = bigpool.tile([128, DK * S], F16, tag="big")
                for j in range(8):
                    flat = fpool.tile([1, 8 * S], F16, tag="flat")
                    nc.sync.dma_start(
                        flat[:], kvts[h][row0 + 8 * j: row0 + 8 * j + 8, :])
                    for half in range(4):
                        d = 8 * j + 2 * half
                        ps = pspool.tile([128, 2 * S], F32, tag="ps")
                        nc.tensor.matmul(ps[:, 0:S], ones[:],
                                         flat[:, 2 * half * S:(2 * half + 1) * S])
                        nc.tensor.matmul(ps[:, S:2 * S], ones[:],
                                         flat[:, (2 * half + 1) * S:(2 * half + 2) * S])
                        nc.scalar.copy(big[:, d * S:(d + 2) * S], ps[:])
                return big

            ctxpairs = []
            for _ch in range(NCH):
                ctxp = ctxpool.tile([CH, HPC * DK], F16, tag="ctxp")
                ctxpairs.append(ctxp)
            scores_tiles = {}
            _bigcm = tc.tile_pool(name="big", bufs=2)
            bigpool = _bigcm.__enter__()
            for h in range(HPC):
                kb = build_bcast(h, 0)      # kT broadcast
                # stage 1: A = max_d(k-q), Bt = min_d(k-q); scores = Bt - A
                for ch in range(NCH):
                    A = aapool.tile([CH, S], F16, tag="A")
                    Bt = bbpool.tile([CH, S], F16, tag="B")
                    qf = qfs[h, ch]
                    nc.vector.tensor_scalar(A[:], kb[:, 0:S], qf[:, 0:1], None,
                                            OP.subtract)
                    nc.vector.tensor_scalar(Bt[:], kb[:, 0:S], qf[:, 0:1], None,
                                            OP.subtract)
                    for d in range(1, DK):
                        kbd = kb[:, d * S:(d + 1) * S]
                        qcol = qf[:, d:d + 1]
                        nc.vector.scalar_tensor_tensor(
                            A[:], kbd, qcol, A[:], OP.subtract, OP.max)
                        nc.vector.scalar_tensor_tensor(
                            Bt[:], kbd, qcol, Bt[:], OP.subtract, OP.min)
                    sc = scpool.tile([CH, S], F16, tag="sc")
                    nc.vector.tensor_tensor(sc[:], Bt[:], A[:], OP.subtract)
                    scores_tiles[h, ch] = sc

                vb = build_bcast(h, DK)     # vT broadcast
                # stage 2: ctx[c, e] = max_s(scores[c,s] + v[s,e])
                # (tensor_tensor_reduce crashes TRN2 here; use TT add +
                #  tensor_reduce max instead)
                for ch in range(NCH):
                    sc = scores_tiles[h, ch]
                    for e in range(DK):
                        scr = scrpool.tile([CH, S], F16, tag="scr")
                        nc.vector.tensor_tensor(
                            scr[:], sc[:], vb[:, e * S:(e + 1) * S], OP.add)
                        nc.vector.tensor_reduce(
                            ctxpairs[ch][:, h * DK + e: h * DK + e + 1],
                            scr[:], axis=mybir.AxisListType.X, op=OP.max)

            _bigcm.__exit__(None, None, None)
            # projection: partial[ch] = (exp(ctx)-1).T-matmul with wo, then
            # fp16 ReduceScatter over the 4-core batch group -> this core's
            # 128-row slice of the final output.
            partial = dpool.tile([S, DM], F16, tag="partial")
            red = dpool.tile([CH, DM], F16, tag="red")
            for ch in range(NCH):
                eT = projpool.tile([128, 128], F16, tag="eT")
                nc.sync.dma_start(eT[:], ctxpairs[ch][:], transpose=True)
                ex = projpool.tile([128, 128], F16, tag="ex")
                nc.scalar.activation(ex[:], eT[:], AF.Exp)
                nc.vector.tensor_scalar(ex[:], ex[:], -1.0, None, OP.add)
                pso = psopool.tile([128, DM], F32, tag="pso")
                nc.tensor.matmul(pso[:], ex[:], wo_sb[:])
                osb = projpool.tile([128, DM], F16, tag="osb")
                nc.scalar.copy(osb[:], pso[:])
                nc.sync.dma_start(partial[ch * CH:(ch + 1) * CH, :], osb[:])
            nc.gpsimd.collective_compute(
                "ReduceScatter",
                mybir.AluOpType.add,
                replica_groups=[[0, 1, 2, 3], [4, 5, 6, 7]],
                ins=[partial.opt()],
                outs=[red.opt()],
            )
            nc.gpsimd.dma_start(outp[:], red[:])

    nc.compile()
    return nc


def _prep_global(x, Wq, Wk, Wv, W_out):
    """fp16 host-side packing, already concatenated along the core axis."""
    NW = DK * 3 * DK
    x16 = np.asarray(x, np.float32).astype(np.float16)          # (B,S,DM)
    # core c=(b*4+hp) gets x[b, :, 128*hp : 128*hp+128]
    xh_g = np.ascontiguousarray(
        x16.reshape(B, S, 4, HPC * DK).transpose(0, 2, 1, 3)
        .reshape(NCORES * S, HPC * DK))
    wcat16 = np.concatenate(
        [np.asarray(Wq).T, np.asarray(Wk).T, np.asarray(Wv).T],
        axis=1).astype(np.float16)                              # (64, 192)
    wcat_g = np.ascontiguousarray(wcat16.reshape(NCORES, NW // NCORES))
    wot16 = np.asarray(W_out, np.float32).T.astype(np.float16)  # (DM, DM)
    # core c=(b*4+hp) carries rows [128*hp + 64*b, +64) of W_out^T
    wo_g = np.ascontiguousarray(
        wot16.reshape(4, 2, DK, DM).transpose(1, 0, 2, 3)
        .reshape(NCORES * DK, DM))
    return {"xh": xh_g, "wcat_in": wcat_g, "wo_in": wo_g}


_runner = None


def _make_runner(nc):
    """Build the shard_map-jitted executable ONCE (mirrors the multi-core
    path of bass2jax.run_bass_via_pjrt) so repeat calls skip re-tracing.

    Output placeholder operands are persistent device-resident arrays and are
    NOT donated: the NEFF writes every element of its outputs, so the
    placeholder content is never read and no per-call host->device upload of
    zero buffers is needed."""
    import jax
    import numpy as _np
    from concourse import mybir
    from concourse.bass2jax import (
        Mesh, PartitionSpec, _bass_exec_p, install_neuronx_cc_hook,
        partition_id_tensor, shard_map,
    )

    install_neuronx_cc_hook()
    partition_name = (nc.partition_id_tensor.name
                      if nc.partition_id_tensor else None)
    in_names, out_names, out_avals, zero_outs = [], [], [], []
    for alloc in nc.m.functions[0].allocations:
        if not isinstance(alloc, mybir.MemoryLocationSet):
            continue
        name = alloc.memorylocations[0].name
        if alloc.kind == "ExternalInput":
            if name != partition_name:
                in_names.append(name)
        elif alloc.kind == "ExternalOutput":
            shape = tuple(alloc.tensor_shape)
            dtype = mybir.dt.np(alloc.dtype)
            out_avals.append(jax.core.ShapedArray(shape, dtype))
            out_names.append(name)
            zero_outs.append(_np.zeros(shape, dtype))
    n_params = len(in_names)
    all_names = list(in_names) + list(out_names)
    if partition_name is not None:
        all_names.append(partition_name)

    def _body(*args):
        operands = list(args)
        if partition_name is not None:
            operands.append(partition_id_tensor())
        return tuple(_bass_exec_p.bind(
            *operands, out_avals=tuple(out_avals), in_names=tuple(all_names),
            out_names=tuple(out_names), lowering_input_output_aliases=(),
            sim_require_finite=True, sim_require_nnan=True, nc=nc))

    devices = jax.devices()[:NCORES]
    mesh = Mesh(_np.asarray(devices), ("core",))
    n_outs = len(out_avals)
    in_specs = (PartitionSpec("core"),) * (n_params + n_outs)
    out_specs = (PartitionSpec("core"),) * n_outs
    sharded = jax.jit(
        shard_map(_body, mesh=mesh, in_specs=in_specs, out_specs=out_specs,
                  check_rep=False),
        keep_unused=True)

    # one-time upload of the (never-read) output placeholder buffers
    out_sharding = jax.sharding.NamedSharding(mesh, PartitionSpec("core"))
    placeholders = [
        jax.device_put(
            _np.zeros((NCORES * z.shape[0], *z.shape[1:]), z.dtype),
            out_sharding)
        for z in zero_outs]
    jax.block_until_ready(placeholders)

    def run(global_in):
        concat_in = [global_in[nm] for nm in in_names]
        out_arrs = sharded(*concat_in, *placeholders)
        return {nm: _np.asarray(out_arrs[i]) for i, nm in enumerate(out_names)}

    return run


def kernel(x, Wq, Wk, Wv, W_out):
    global _prog, _runner
    if _prog is None:
        _prog = _build_program()
    if _runner is None:
        _runner = _make_runner(_prog)

    results = _runner(_prep_global(x, Wq, Wk, Wv, W_out))

    # outp global is (8*128, 512) fp16; core c=(b*4+hp) holds rows
    # [128*hp, 128*(hp+1)) of batch b
    outp = results["outp"].reshape(B, 4, CH, DM)
    return np.ascontiguousarray(outp.reshape(B, S, DM), dtype=np.float32)


def time_device(x, Wq, Wk, Wv, W_out, n=10):
    """Min wall-clock of a warm full call (prep + upload + exec + fetch)."""
    import time as _t
    global _prog, _runner
    if _prog is None:
        _prog = _build_program()
    if _runner is None:
        _runner = _make_runner(_prog)
    args = (x, Wq, Wk, Wv, W_out)
    _runner(_prep_global(*args))  # warm
    t1 = []
    for _ in range(n):
        t0 = _t.perf_counter()
        _runner(_prep_global(*args))
        t1.append(_t.perf_counter() - t0)
    return min(t1) * 1e9, min(t1) * 1e9
